# revision 1
# baseline (speedup 1.0000x reference)
"""Causal self-attention (B=4, T=2048, C=1024, H=16) on 8 trn2 NeuronCores.

Sharding: core c handles batch b = c//2 and head-group hg = c%2 (8 heads).
Each core computes qkv for its heads, causal attention, and the partial
output projection y_hg @ W_proj[hg*512:(hg+1)*512, :].  The Megatron-style
all-reduce after c_proj is done on the host (sum of 2 partials per batch).

v3: software-pipelined emission.  The attention inner loop (scores one
key-block ahead -> exp -> p@v in [queries, dims] orientation) is the
spine; dense matmul groups (qkv projections, output projection) are
pumped into its bubbles so the PE never idles while the scalar engine
exps.  Softmax rowsums ride along as a ones-column of v; normalization
is a per-partition reciprocal + broadcast multiply; y returns to
feature-major via one xbar-transpose DMA per (head-pair, chunk).
"""

import sys

sys.path.insert(0, "/opt/trn_rl_repo")

import numpy as np
import ml_dtypes

B, T, C = 4, 2048, 1024
H = 16          # total heads
HL = 8          # heads per core
D = 64          # head dim
HG = HL * D     # 512, per-core qkv feature width
KB = C // 128   # 8 k-blocks over the contraction dim C
PB = HG // 128  # 4 head-pair blocks (128 features each)
NCI = T // 512  # 4 query chunks

_PROGRAM = None


def _build_program(legalize=True):
    import concourse.bass as bass
    import concourse.tile as tile
    from concourse import mybir

    bf16 = mybir.dt.bfloat16
    f32 = mybir.dt.float32
    Act = mybir.ActivationFunctionType
    Alu = mybir.AluOpType

    nc = bass.Bass()

    x_d = nc.dram_tensor("x", [T, C], bf16, kind="ExternalInput")
    wq_d = nc.dram_tensor("wq", [C, HG], bf16, kind="ExternalInput")
    wk_d = nc.dram_tensor("wk", [C, HG], bf16, kind="ExternalInput")  # pre-scaled by 1/8
    wv_d = nc.dram_tensor("wv", [C, HG], bf16, kind="ExternalInput")
    bq_d = nc.dram_tensor("bq", [HG], f32, kind="ExternalInput")
    bk_d = nc.dram_tensor("bk", [HG], f32, kind="ExternalInput")  # pre-scaled by 1/8
    bv_d = nc.dram_tensor("bv", [HG], f32, kind="ExternalInput")
    wp_d = nc.dram_tensor("wp", [HG, C], bf16, kind="ExternalInput")
    mask_d = nc.dram_tensor("mask", [128, 128], bf16, kind="ExternalInput")
    out_d = nc.dram_tensor("out", [T, C], bf16, kind="ExternalOutput")

    with tile.TileContext(nc) as tc:
        with (
            tc.tile_pool(name="const", bufs=1) as const,
            tc.tile_pool(name="big", bufs=1) as big,
            tc.tile_pool(name="work", bufs=18) as work,
            tc.tile_pool(name="ypr", bufs=2) as ypr,
            tc.tile_pool(name="recp", bufs=2) as recp,
            tc.tile_pool(name="outp", bufs=2) as outp,
            tc.tile_pool(name="ps_blk", bufs=2, space="PSUM") as ps_blk,
            tc.tile_pool(name="ps_st", bufs=2, space="PSUM") as ps_st,
            tc.tile_pool(name="ps_y", bufs=1, space="PSUM") as ps_y,
        ):
            # ---- DMA order: tiny constants, then x/weight chunks in
            # first-use order so the first matmuls start within ~1us ----
            mask_sb = const.tile([128, 128], bf16)
            nc.sync.dma_start(mask_sb[:], mask_d[:])
            bv_bc = const.tile([128, HG], f32)
            bv_ap = bv_d.ap()
            nc.sync.dma_start(
                bv_bc[:],
                bass.AP(tensor=bv_ap.tensor, offset=bv_ap.offset, ap=[[0, 128], *bv_ap.ap]),
            )
            bq_sb = const.tile([128, PB], f32)
            nc.sync.dma_start(bq_sb[:], bq_d.ap().rearrange("(o p) -> p o", p=128))
            bk_sb = const.tile([128, PB], f32)
            nc.sync.dma_start(bk_sb[:], bk_d.ap().rearrange("(o p) -> p o", p=128))

            xt = big.tile([128, KB, T], bf16)
            wv_sb = big.tile([128, KB, HG], bf16)
            wq_sb = big.tile([128, KB, HG], bf16)
            wk_sb = big.tile([128, KB, HG], bf16)
            nc.sync.dma_start(wv_sb[:], wv_d.ap().rearrange("(ko p) n -> p ko n", p=128))
            for k in range(KB):
                nc.sync.dma_start_transpose(
                    xt[:, k, 0:512], x_d[0:512, k * 128 : (k + 1) * 128]
                )
            nc.sync.dma_start(wq_sb[:], wq_d.ap().rearrange("(ko p) n -> p ko n", p=128))
            nc.sync.dma_start(wk_sb[:], wk_d.ap().rearrange("(ko p) n -> p ko n", p=128))
            for ci in range(1, NCI):
                for k in range(KB):
                    nc.sync.dma_start_transpose(
                        xt[:, k, ci * 512 : (ci + 1) * 512],
                        x_d[ci * 512 : (ci + 1) * 512, k * 128 : (k + 1) * 128],
                    )
                if ci == 1:
                    wp_sb = big.tile([128, PB, C], bf16)
                    nc.sync.dma_start(
                        wp_sb[:], wp_d.ap().rearrange("(ko p) n -> p ko n", p=128)
                    )

            # persistent activations
            qt = big.tile([128, PB, T], bf16)   # q^T: block m = heads 2m,2m+1
            kt = big.tile([128, PB, T], bf16)   # k^T (pre-scaled by 1/8 via wk)
            va = big.tile([128, T // 128, HL, D + 1], bf16)  # v rows + ones col
            yt = big.tile([128, PB, T], bf16)   # y^T (normalized)

            nc.vector.memset(va[:, :, :, D : D + 1], 1.0)

            # ---- dense work groups (one PSUM accumulation each) ----
            def v_group(jb):
                vsl = slice(jb * 128, (jb + 1) * 128)
                v_ps = ps_blk.tile([128, 512], f32, tag="blk", name=f"vps_{jb}")
                for k in range(KB):
                    nc.tensor.matmul(
                        v_ps[:],
                        xt[:, k, vsl],
                        wv_sb[:, k, :],
                        start=(k == 0),
                        stop=(k == KB - 1),
                    )
                nc.vector.tensor_tensor(
                    va[:, jb, :, 0:D],
                    v_ps[:].rearrange("p (h d) -> p h d", h=HL),
                    bv_bc[:].rearrange("p (h d) -> p h d", h=HL),
                    Alu.add,
                )

            def q_group(m, ci):
                tsl = slice(ci * 512, (ci + 1) * 512)
                q_ps = ps_blk.tile([128, 512], f32, tag="blk", name=f"qps_{m}_{ci}")
                for k in range(KB):
                    nc.tensor.matmul(
                        q_ps[:],
                        wq_sb[:, k, m * 128 : (m + 1) * 128],
                        xt[:, k, tsl],
                        start=(k == 0),
                        stop=(k == KB - 1),
                    )
                nc.vector.tensor_scalar(
                    qt[:, m, tsl], q_ps[:], bq_sb[:, m : m + 1], None, Alu.add
                )

            def k_group(m, ci):
                tsl = slice(ci * 512, (ci + 1) * 512)
                k_ps = ps_blk.tile([128, 512], f32, tag="blk", name=f"kps_{m}_{ci}")
                for k in range(KB):
                    nc.tensor.matmul(
                        k_ps[:],
                        wk_sb[:, k, m * 128 : (m + 1) * 128],
                        xt[:, k, tsl],
                        start=(k == 0),
                        stop=(k == KB - 1),
                    )
                nc.vector.tensor_scalar(
                    kt[:, m, tsl], k_ps[:], bk_sb[:, m : m + 1], None, Alu.add
                )

            def proj_group(rb):
                # one 128-row output block: y[rb] @ W_proj -> DMA out
                tsl = slice(rb * 128, (rb + 1) * 128)
                ot = outp.tile([128, C], bf16, tag="ot", name=f"ot_{rb}")
                for n2 in range(C // 512):
                    o_ps = ps_blk.tile([128, 512], f32, tag="blk", name=f"ops_{rb}_{n2}")
                    for kb in range(PB):
                        nc.tensor.matmul(
                            o_ps[:],
                            yt[:, kb, tsl],
                            wp_sb[:, kb, n2 * 512 : (n2 + 1) * 512],
                            start=(kb == 0),
                            stop=(kb == PB - 1),
                        )
                    nc.vector.tensor_copy(ot[:, n2 * 512 : (n2 + 1) * 512], o_ps[:])
                nc.sync.dma_start(out_d[tsl, :], ot[:])

            # ---- pump queue: (earliest_window, closure) consumed in order;
            # barrier[w] = all groups with before<=w must be emitted before
            # window w's attention starts.  Window index w = 4*ci + m. ----
            queue = []

            def add(before, earliest, fn):
                queue.append([before, earliest, fn])

            # pre-loop (emitted directly): v(0..3), qk(0,0)
            # pumped: everything else, as late as dependencies allow
            for m in range(1, 4):
                add(4 * 0 + m, 0, (lambda mm: lambda: q_group(mm, 0))(m))
                add(4 * 0 + m, 0, (lambda mm: lambda: k_group(mm, 0))(m))
            for ci in range(1, NCI):
                # xt chunk ci lands after ~(4+ci) MB of input DMA; don't pump
                # its consumers before then or the PE parks on the DMA queue
                dma_ear = 4 * (ci - 1) + 2
                for g in range(4):
                    add(4 * ci + 0, max(dma_ear, 4 * ci - 2),
                        (lambda jb: lambda: v_group(jb))(4 * ci + g))
                for m in range(4):
                    bef = 4 * ci + m
                    ear = max(dma_ear, bef - 1)
                    add(bef, ear, (lambda mm, cc: lambda: q_group(mm, cc))(m, ci))
                    add(bef, ear, (lambda mm, cc: lambda: k_group(mm, cc))(m, ci))
            for ci in range(NCI):
                for g in range(4):
                    # proj of chunk ci: yt(·, ci) complete after window 4*ci+3,
                    # so earliest consumer window is 4*ci+4 (plus xbar slack)
                    add(99, 4 * ci + 4, (lambda rb: lambda: proj_group(rb))(4 * ci + g))

            qpos = [0]

            def drain_until(w):
                while qpos[0] < len(queue) and queue[qpos[0]][0] <= w:
                    queue[qpos[0]][2]()
                    qpos[0] += 1

            def pump(w):
                if qpos[0] < len(queue) and queue[qpos[0]][1] <= w:
                    queue[qpos[0]][2]()
                    qpos[0] += 1
                    return True
                return False

            # ---- attention spine ----
            def attn(m, ci):
                w = 4 * ci + m
                njb = 4 * ci + 4
                budget = njb // 3 + 1
                yp = [
                    ps_y.tile([128, 4, D + 1], f32, tag=f"y{par}", name=f"yp{par}_{m}_{ci}")
                    for par in (0, 1)
                ]
                sts = {}
                pts = {}

                def s_emit(jb):
                    o = max(0, 128 * jb - 512 * ci)
                    wdt = 512 - o
                    st = ps_st.tile([128, 1024], f32, tag="st", name=f"st_{m}_{ci}_{jb}")
                    sts[jb] = st
                    for par in (0, 1):
                        p0 = 64 * par
                        nc.tensor.matmul(
                            st[:, 512 * par : 512 * par + wdt],
                            kt[p0 : p0 + 64, m, 128 * jb : 128 * (jb + 1)],
                            qt[p0 : p0 + 64, m, 512 * ci + o : 512 * ci + 512],
                            start=True,
                            stop=True,
                        )

                def e_emit(jb):
                    o = max(0, 128 * jb - 512 * ci)
                    wdt = 512 - o
                    st = sts.pop(jb)
                    pt = work.tile([128, 1024], bf16, tag="pt", name=f"pt_{m}_{ci}_{jb}")
                    pts[jb] = pt
                    if wdt == 512:
                        nc.scalar.activation(pt[:], st[:], Act.Exp)
                    else:
                        nc.scalar.activation(pt[:, 0:wdt], st[:, 0:wdt], Act.Exp)
                        nc.scalar.activation(
                            pt[:, 512 : 512 + wdt], st[:, 512 : 512 + wdt], Act.Exp
                        )
                    if jb >= 4 * ci:
                        # zero the sub-diagonal triangle post-exp (on GpSimd:
                        # SBUF-only op, keeps the DVE queue short); one strided
                        # op covers both heads' diagonal chunks
                        pp = pt[:]
                        diag = bass.AP(
                            tensor=pp.tensor,
                            offset=pp.offset,
                            ap=[pp.ap[0], [512, 2], [1, 128]],
                        )
                        mb = mask_sb[:]
                        mask2 = bass.AP(
                            tensor=mb.tensor,
                            offset=mb.offset,
                            ap=[mb.ap[0], [0, 2], [1, 128]],
                        )
                        nc.gpsimd.tensor_tensor(diag, diag, mask2, Alu.mult)

                pumped = 0
                # score/exp stream first (pipelined with pumped dense work),
                # then p@v groups strictly region-sequential: interleaved
                # accumulation groups within one PSUM bank lose their
                # has_written state (second start=True clobbers the first
                # group's bits), so group (par, qb) runs start->stop alone.
                for jb in range(njb):
                    s_emit(jb)
                    e_emit(jb)
                    if pumped < budget and pump(w):
                        pumped += 1
                for qb in range(4):
                    if pump(w):
                        pumped += 1
                    for par in (0, 1):
                        for jb in range(4 * ci + qb + 1):
                            o = max(0, 128 * jb - 512 * ci)
                            lo = qb * 128 - o
                            nc.tensor.matmul(
                                yp[par][:, qb, :],
                                pts[jb][:, 512 * par + lo : 512 * par + lo + 128],
                                va[:, jb, 2 * m + par, :],
                                start=(jb == 0),
                                stop=(jb == 4 * ci + qb),
                            )
                # normalize: y /= rowsum, then one xbar transpose into y^T
                yy = ypr.tile([128, 4, 2, D], bf16, tag="yy", name=f"yy_{m}_{ci}")
                for par in (0, 1):
                    rec = recp.tile([128, 4, 1], f32, tag=f"rec{par}")
                    nc.vector.reciprocal(rec[:], yp[par][:, :, D : D + 1])
                    rr = rec[:]
                    rec_bc = bass.AP(
                        tensor=rr.tensor,
                        offset=rr.offset,
                        ap=[rr.ap[0], rr.ap[1], [0, D]],
                    )
                    nc.vector.tensor_tensor(
                        yy[:, :, par, :], yp[par][:, :, 0:D], rec_bc, Alu.mult
                    )
                nc.sync.dma_start_transpose(
                    yt[:, m, ci * 512 : (ci + 1) * 512].rearrange(
                        "p (b q) -> p b q", b=4
                    ),
                    yy[:].rearrange("p a b c -> p (a b c)"),
                )

            # ---- top level ----
            for jb in range(4):
                v_group(jb)
            q_group(0, 0)
            k_group(0, 0)
            for ci in range(NCI):
                for m in range(PB):
                    drain_until(4 * ci + m)
                    attn(m, ci)
            while qpos[0] < len(queue):
                queue[qpos[0]][2]()
                qpos[0] += 1

    nc.finalize()
    if legalize:
        _legalize_waits(nc, mybir)
    return nc


def _legalize_waits(nc, mybir):
    """This walrus build only encodes 1 wait + 1 update per engine ISA
    instruction; hoist extra waits onto preceding same-engine NoOps (and
    extra updates onto following NoOps).  Engines execute in-order and
    waits only reference earlier-scheduled producers, so this is sound."""
    ctr = 0
    for blk in nc.m.functions[0].blocks:
        insts = list(blk.instructions)
        out = []
        changed = False
        for inst in insts:
            si = inst.sync_info
            waits = list(si.on_wait) if (si and si.on_wait) else []
            upds = list(si.on_update) if (si and si.on_update) else []
            if len(waits) > 1:
                for w in waits[:-1]:
                    ctr += 1
                    nop = mybir.InstNoOp(name=f"I-wsplit-{ctr}", engine=inst.engine)
                    nop.sync_info = mybir.SyncInfo(on_wait=[w], on_update=[])
                    out.append(nop)
                inst.sync_info = mybir.SyncInfo(on_wait=[waits[-1]], on_update=upds)
                changed = True
            out.append(inst)
            if len(upds) > 1:
                inst.sync_info = mybir.SyncInfo(
                    on_wait=list(inst.sync_info.on_wait or []), on_update=[upds[0]]
                )
                for u in upds[1:]:
                    ctr += 1
                    nop = mybir.InstNoOp(name=f"I-usplit-{ctr}", engine=inst.engine)
                    nop.sync_info = mybir.SyncInfo(on_wait=[], on_update=[u])
                    out.append(nop)
                changed = True
        if changed:
            blk.instructions = out


def _get_program():
    global _PROGRAM
    if _PROGRAM is None:
        _PROGRAM = _build_program()
    return _PROGRAM


def _make_in_maps(x, W_attn, b_attn, W_proj):
    bf = ml_dtypes.bfloat16
    x = np.asarray(x, dtype=np.float32)
    W_attn = np.asarray(W_attn, dtype=np.float32)
    b_attn = np.asarray(b_attn, dtype=np.float32)

    mask = (
        np.arange(128)[None, :] >= np.arange(128)[:, None]
    ).astype(ml_dtypes.bfloat16)

    in_maps = []
    for core in range(8):
        b, hg = core // 2, core % 2
        qs = slice(hg * HG, (hg + 1) * HG)
        ks = slice(C + hg * HG, C + (hg + 1) * HG)
        vs = slice(2 * C + hg * HG, 2 * C + (hg + 1) * HG)
        in_maps.append(
            {
                "x": x[b].astype(bf),
                "wq": W_attn[:, qs].astype(bf),
                "wk": (W_attn[:, ks] * 0.125).astype(bf),
                "wv": W_attn[:, vs].astype(bf),
                "bq": b_attn[qs].astype(np.float32),
                "bk": (b_attn[ks] * 0.125).astype(np.float32),
                "bv": b_attn[vs].astype(np.float32),
                "wp": np.asarray(W_proj, dtype=np.float32)[qs, :].astype(bf),
                "mask": mask,
            }
        )
    return in_maps


def run_sharded(x, W_attn, b_attn, W_proj, b_proj, trace=False):
    from concourse.bass_utils import run_bass_kernel_spmd

    nc = _get_program()
    in_maps = _make_in_maps(x, W_attn, b_attn, W_proj)
    res = run_bass_kernel_spmd(nc, in_maps, core_ids=list(range(8)), trace=trace)
    outs = [np.asarray(r["out"], dtype=np.float32) for r in res.results]
    b_proj = np.asarray(b_proj, dtype=np.float32)
    y = np.stack([outs[2 * b] + outs[2 * b + 1] for b in range(B)]) + b_proj
    return y.astype(np.float32), res


def kernel(x, W_attn, b_attn, W_proj, b_proj, train=0, **_kw):
    y, _ = run_sharded(x, W_attn, b_attn, W_proj, b_proj, trace=False)
    return y


def bench_exec(x, W_attn, b_attn, W_proj, iters=20):
    """Steady-state device execution timing: inputs committed to devices once,
    then `iters` chained executions (no donation, outputs stay on device)."""
    import time

    import jax
    import numpy as np
    from jax.sharding import Mesh, PartitionSpec
    from jax.experimental.shard_map import shard_map

    from concourse import bass2jax, mybir
    from concourse.bass2jax import _bass_exec_p, install_neuronx_cc_hook, partition_id_tensor

    nc = _get_program()
    in_maps = _make_in_maps(x, W_attn, b_attn, W_proj)
    n_cores = 8
    install_neuronx_cc_hook()

    partition_name = nc.partition_id_tensor.name if nc.partition_id_tensor else None
    in_names, out_names, out_avals, zero_outs = [], [], [], []
    for alloc in nc.m.functions[0].allocations:
        if not isinstance(alloc, mybir.MemoryLocationSet):
            continue
        name = alloc.memorylocations[0].name
        if alloc.kind == "ExternalInput":
            if name != partition_name:
                in_names.append(name)
        elif alloc.kind == "ExternalOutput":
            shape = tuple(alloc.tensor_shape)
            dtype = mybir.dt.np(alloc.dtype)
            out_names.append(name)
            out_avals.append(jax.core.ShapedArray(shape, dtype))
            zero_outs.append(np.zeros(shape, dtype))
    if nc.dbg_addr is not None:
        in_maps = [
            {**m, nc.dbg_addr.name: np.zeros((1, 2), np.uint32)} for m in in_maps
        ]
        if nc.dbg_addr.name not in in_names:
            in_names.append(nc.dbg_addr.name)
    n_params = len(in_names)
    all_in = list(in_names) + list(out_names)
    if partition_name is not None:
        all_in.append(partition_name)

    def _body(*args):
        operands = list(args)
        if partition_name is not None:
            operands.append(partition_id_tensor())
        outs = _bass_exec_p.bind(
            *operands,
            out_avals=tuple(out_avals),
            in_names=tuple(all_in),
            out_names=tuple(out_names),
            lowering_input_output_aliases=(),
            sim_require_finite=True,
            sim_require_nnan=True,
            nc=nc,
        )
        return tuple(outs)

    devices = jax.devices()[:n_cores]
    mesh = Mesh(np.asarray(devices), ("core",))
    in_specs = (PartitionSpec("core"),) * (n_params + len(out_names))
    out_specs = (PartitionSpec("core"),) * len(out_names)
    fn = jax.jit(
        shard_map(_body, mesh=mesh, in_specs=in_specs, out_specs=out_specs, check_rep=False),
        keep_unused=True,
    )
    concat_in = [
        np.concatenate([np.asarray(in_maps[c][nm]) for c in range(n_cores)], axis=0)
        for nm in in_names
    ]
    concat_zeros = [
        np.zeros((n_cores * z.shape[0], *z.shape[1:]), z.dtype) for z in zero_outs
    ]
    from jax.sharding import NamedSharding

    sh = NamedSharding(mesh, PartitionSpec("core"))
    dev_in = [jax.device_put(a, sh) for a in concat_in]
    dev_zeros = [jax.device_put(a, sh) for a in concat_zeros]
    # warmup (compile + first exec)
    out = fn(*dev_in, *dev_zeros)
    jax.block_until_ready(out)
    times = []
    for _ in range(3):
        t0 = time.perf_counter()
        outs = [fn(*dev_in, *dev_zeros) for _ in range(iters)]
        jax.block_until_ready(outs)
        t1 = time.perf_counter()
        times.append((t1 - t0) / iters)
    return min(times)

